# revision 8
# baseline (speedup 1.0000x reference)
"""Trainium2 Bass kernel for nn_CrossAttention (B=2, N=2048, C=1024, H=16).

Sharding: 16 heads / 8 cores = 2 heads per core (both batches on every
core).  Each core computes its heads' Q/K/V projections with the matching
128-row slice of Wq/Wk/Wv, full attention for its 4 (batch, head) pairs,
and a partial output projection against its 128-column slice of Wp.  The
host sums the 8 partial projections (the tensor-parallel all-reduce) and
adds the bias.

v2 schedule: the ScalarE exp of 16.8M score elements (~147us at
[128,1024] grain) is the pacing engine; everything else is organized so
ScalarE never waits.  The kernel runs four "score units" (b, qp), each
16 m-iterations of: 4 scores matmuls (both heads, PE row-tiled) -> 2 exps
-> (delayed-one-unit) attn@V col-tiled pairs -> (lagged-1) denominator
quad -> one woven misc op (qkv projection half / V transpose batch /
output-projection piece).

PSUM map (8 banks of 2KB/partition):
  sc pool  2 x [128,1024] f32 = 4 banks  (scores ping-pong)
  o  pool  1 x [128,1024] f32 = 2 banks  (attnv accum; qkv quarters in A)
  d  pool  1 x [128, 512] f32 = 1 bank   (softmax denominators)
  m  pool  1 x [128, 512] f32 = 1 bank   (qkv halves / transposes / proj)

On-device layouts (per core, fp16 matmul operands):
  xq/xt    [1024 ch, 4096 pos]  channel-major inputs (host pre-transposed)
  q2T/k2T/v2T  4 x [128, 1024]  per (batch, pos-half); rows 0-63 head0
  vpos     2 x [128, 2048]      per batch: [keys-in-chunk, chunk*128 + hd]
  st       [128 keys, 2048]     exp(scores^T) per (b, qp, m): h0 | h1
  outT     4 x [128, 1024]      normalized attention output per (b, qp)
  out_p    [2, 2048, 1024] f32  partial projection (summed on host)
"""

import os
import sys

for _p in ("/opt/trn_rl_repo", os.path.expanduser("~/.axon_site/_ro/trn_rl_repo")):
    if os.path.isdir(_p) and _p not in sys.path:
        sys.path.insert(0, _p)

import numpy as np

import concourse.bacc as bacc
import concourse.mybir as mybir
import concourse.tile as tile
from concourse.bass_utils import run_bass_kernel_spmd

F16 = mybir.dt.float16
F32 = mybir.dt.float32
AF = mybir.ActivationFunctionType

B, N, C, H, D = 2, 2048, 1024, 16, 64
NCORES = 8
SCALE = float(D) ** -0.5

TRACE = False
LAST_EXEC_NS = None
LAST_RESULTS = None

_COMPILED_NC = None


def _emit(nc):
    xq = nc.dram_tensor("xq", [C, B * N], F16, kind="ExternalInput")
    xt = nc.dram_tensor("xt", [C, B * N], F16, kind="ExternalInput")
    wq = nc.dram_tensor("wq", [128, 1024], F16, kind="ExternalInput")
    wk = nc.dram_tensor("wk", [128, 1024], F16, kind="ExternalInput")
    wv = nc.dram_tensor("wv", [128, 1024], F16, kind="ExternalInput")
    wp = nc.dram_tensor("wp", [128, C], F16, kind="ExternalInput")
    id128 = nc.dram_tensor("id128", [128, 128], F16, kind="ExternalInput")
    outp = nc.dram_tensor("out_p", [B, N, C], F32, kind="ExternalOutput")

    with tile.TileContext(nc) as tc:
        with (
            tc.tile_pool(name="consts", bufs=1) as cpool,
            tc.tile_pool(name="xs", bufs=8) as xs,
            tc.tile_pool(name="big", bufs=1) as big,
            tc.tile_pool(name="stp", bufs=19) as stp,
            tc.tile_pool(name="ob", bufs=1) as obp,
            tc.tile_pool(name="rcp", bufs=2) as rcpool,
            tc.tile_pool(name="evo", bufs=2) as evop,
            tc.tile_pool(name="pev", bufs=6) as pevp,
            tc.tile_pool(name="sc", bufs=2, space="PSUM") as scp,
            tc.tile_pool(name="o", bufs=1, space="PSUM") as op,
            tc.tile_pool(name="d", bufs=1, space="PSUM") as dp,
            tc.tile_pool(name="m", bufs=1, space="PSUM") as mp,
        ):
            # ---- constants ------------------------------------------------
            w_sb = {}
            for name, dram in (("wq", wq), ("wk", wk), ("wv", wv)):
                t_ = cpool.tile([128, 1024], F16, tag=name, name=f"w_{name}")
                nc.sync.dma_start(t_[:], dram[:])
                w_sb[name] = t_
            wp_sb = cpool.tile([128, C], F16, tag="wp")
            id_sb = cpool.tile([128, 128], F16, tag="id128")
            ones_sb = cpool.tile([128, 1], F16, tag="ones")
            nc.vector.memset(ones_sb[:], 1.0)

            def late_consts():
                nc.sync.dma_start(id_sb[:], id128[:])
                nc.sync.dma_start(wp_sb[:], wp[:])

            q2T = {}
            k2T = {}
            v2T = {}
            vpos = {}
            outT = {}
            for b in range(2):
                vpos[b] = big.tile([128, 2048], F16, tag=f"vpos{b}", name=f"vpos{b}")
                for hf in range(2):
                    q2T[(b, hf)] = big.tile([128, 1024], F16, tag=f"q{b}{hf}", name=f"q2T{b}{hf}")
                    k2T[(b, hf)] = big.tile([128, 1024], F16, tag=f"k{b}{hf}", name=f"k2T{b}{hf}")
                    v2T[(b, hf)] = big.tile([128, 1024], F16, tag=f"v{b}{hf}", name=f"v2T{b}{hf}")
                    outT[(b, hf)] = obp.tile(
                        [128, 1024], F16, tag=f"outT{b}{hf}", name=f"outT{b}{hf}"
                    )

            st = {(b, qp): [] for b in range(2) for qp in range(2)}
            ps_o = {}
            ps_d = {}
            rc_t = {}

            # ---- emission helpers -----------------------------------------
            def half(dst, wname, xdram, b, hf, qc):
                # one [128 rows, 512 pos] qkv projection half via the misc pool
                ps = mp.tile([128, 512], F32, tag="m", name=f"h{wname}{b}{hf}{qc}")
                c0 = b * 2048 + hf * 1024 + qc * 512
                for kc in range(8):
                    x_t = xs.tile([128, 512], F16, tag="x", name=f"x{wname}{b}{hf}{qc}{kc}")
                    nc.sync.dma_start(
                        x_t[:], xdram[kc * 128 : (kc + 1) * 128, c0 : c0 + 512]
                    )
                    nc.tensor.matmul(
                        ps[:],
                        lhsT=w_sb[wname][:, kc * 128 : (kc + 1) * 128],
                        rhs=x_t[:],
                        start=(kc == 0),
                        stop=(kc == 7),
                    )
                nc.vector.tensor_copy(dst[:, qc * 512 : (qc + 1) * 512], ps[:])

            def quarter_po(dst, wname, xdram, b, hf):
                # full [128,1024] quarter through the attnv slot (idle
                # before unit B); generator with 2 yield points so it can
                # be woven one ~1.7us chunk per m-iteration.
                ps = op.tile([128, 1024], F32, tag="o", name=f"qt{wname}{b}{hf}")
                c0 = b * 2048 + hf * 1024
                for kc in range(8):
                    for qc in range(2):
                        x_t = xs.tile(
                            [128, 512], F16, tag="x", name=f"x{wname}{b}{hf}{kc}{qc}"
                        )
                        nc.sync.dma_start(
                            x_t[:],
                            xdram[kc * 128 : (kc + 1) * 128, c0 + qc * 512 : c0 + (qc + 1) * 512],
                        )
                        nc.tensor.matmul(
                            ps[:, qc * 512 : (qc + 1) * 512],
                            lhsT=w_sb[wname][:, kc * 128 : (kc + 1) * 128],
                            rhs=x_t[:],
                            start=(kc == 0),
                            stop=(kc == 7),
                        )
                    if kc == 3:
                        yield
                nc.vector.tensor_copy(dst[:], ps[:])
                yield

            def tbatch(b, quad):
                # transpose 4 key-chunks of v2T into vpos via PE
                ps = mp.tile([128, 512], F16, tag="m", name=f"t{b}{quad}")
                for i in range(4):
                    ch = quad * 4 + i
                    src = v2T[(b, ch // 8)]
                    ks = slice((ch % 8) * 128, (ch % 8 + 1) * 128)
                    nc.tensor.transpose(
                        ps[:, i * 128 : (i + 1) * 128], src[:, ks], id_sb[:]
                    )
                nc.vector.tensor_copy(
                    vpos[b][:, quad * 512 : (quad + 1) * 512], ps[:]
                )

            def scores(b, qp, m):
                th = []
                for h in range(2):
                    th.append(
                        scp.tile([128, 1024], F32, tag="sc", name=f"sc{b}{qp}{m}{h}")
                    )
                kt = k2T[(b, m // 8)]
                ms = slice((m % 8) * 128, (m % 8 + 1) * 128)
                for qc in range(2):
                    cs = slice(qc * 512, (qc + 1) * 512)
                    for h in range(2):
                        hp = slice(h * 64, (h + 1) * 64)
                        nc.tensor.matmul(
                            th[h][:, cs],
                            lhsT=kt[hp, ms],
                            rhs=q2T[(b, qp)][hp, cs],
                            start=True,
                            stop=True,
                        )
                s = stp.tile([128, 2048], F16, tag="st", name=f"st{b}{qp}{m}")
                for h in range(2):
                    nc.scalar.activation(
                        s[:, h * 1024 : (h + 1) * 1024], th[h][:], AF.Exp, scale=SCALE
                    )
                st[(b, qp)].append(s)

            def attnv(b, qp, m):
                if m == 0:
                    ps_o[(b, qp)] = op.tile([128, 1024], F32, tag="o", name=f"o{b}{qp}")
                po = ps_o[(b, qp)]
                s = st[(b, qp)][m]
                kw = dict(start=(m == 0), stop=(m == 15))
                for qc in range(2):
                    cs = slice(qc * 512, (qc + 1) * 512)
                    for h in range(2):
                        nc.tensor.matmul(
                            po[h * 64 : (h + 1) * 64, cs],
                            lhsT=vpos[b][:, m * 128 + h * 64 : m * 128 + (h + 1) * 64],
                            rhs=s[:, h * 1024 + qc * 512 : h * 1024 + (qc + 1) * 512],
                            **kw,
                        )

            def denom(b, qp, m):
                if m == 0:
                    ps_d[(b, qp)] = dp.tile([128, 512], F32, tag="d", name=f"d{b}{qp}")
                pd = ps_d[(b, qp)]
                s = st[(b, qp)][m]
                kw = dict(start=(m == 0), stop=(m == 15))
                for qc in range(2):
                    for h in range(2):
                        row = h * 32 + qc * 64
                        nc.tensor.matmul(
                            pd[row : row + 1, 0:512],
                            lhsT=ones_sb[:, 0:1],
                            rhs=s[:, h * 1024 + qc * 512 : h * 1024 + (qc + 1) * 512],
                            skip_group_check=True,
                            tile_position=(0, row),
                            **kw,
                        )

            def recip(b, qp):
                # frees the pd slot for the next unit's denominators
                rc = rcpool.tile([128, 1024], F32, tag="rc", name=f"rc{b}{qp}")
                rc_t[(b, qp)] = rc
                nc.vector.reciprocal_approx_fast(rc[:, 0:512], ps_d[(b, qp)][:, 0:512])

            def shuffles(b, qp):
                # broadcast per-(h,qc) reciprocal rows across their 64-row spans
                rc = rc_t[(b, qp)]
                bcast = [0] * 32
                c0 = slice(0, 512)
                c1 = slice(512, 1024)
                for dst, srow in ((0, 64), (32, 64), (64, 96), (96, 96)):
                    nc.vector.stream_shuffle(
                        rc[dst : dst + 32, c1], rc[srow : srow + 32, c0], bcast
                    )
                for dst, srow in ((64, 32), (96, 32), (32, 0), (0, 0)):
                    nc.vector.stream_shuffle(
                        rc[dst : dst + 32, c0], rc[srow : srow + 32, c0], bcast
                    )

            def po_evict(b, qp):
                ev = evop.tile([128, 1024], F32, tag="evo", name=f"evo{b}{qp}")
                nc.vector.tensor_copy(ev[:], ps_o[(b, qp)][:])
                ps_o[(b, qp)] = ev

            def mul(b, qp):
                nc.vector.tensor_mul(
                    outT[(b, qp)][:], ps_o[(b, qp)][:], rc_t[(b, qp)][:]
                )

            def proj_sub(b, qp, pm, ncol, use_act=False, pool=None, ptag="m"):
                # one [128 pos, 512 outcols] piece of the output projection
                ps = (pool or mp).tile([128, 512], F32, tag=ptag, name=f"pp{b}{qp}{pm}{ncol}")
                nc.tensor.matmul(
                    ps[:],
                    lhsT=outT[(b, qp)][:, pm * 128 : (pm + 1) * 128],
                    rhs=wp_sb[:, ncol * 512 : (ncol + 1) * 512],
                    start=True,
                    stop=True,
                )
                ev = pevp.tile([128, 512], F32, tag="pev", name=f"pe{b}{qp}{pm}{ncol}")
                if use_act:
                    nc.scalar.copy(ev[:], ps[:])
                else:
                    nc.vector.tensor_copy(ev[:], ps[:])
                rows = slice(qp * 1024 + pm * 128, qp * 1024 + (pm + 1) * 128)
                nc.sync.dma_start(
                    outp[b, rows, ncol * 512 : (ncol + 1) * 512], ev[:]
                )

            # ---- weave generators -----------------------------------------
            def weave_A():
                # attnv slot is idle in unit A: qkv quarters run through it
                yield from quarter_po(v2T[(0, 0)], "wv", xt, 0, 0)
                yield from quarter_po(q2T[(0, 1)], "wq", xq, 0, 1)
                yield from quarter_po(k2T[(0, 1)], "wk", xt, 0, 1)
                yield from quarter_po(v2T[(0, 1)], "wv", xt, 0, 1)
                for quad in range(4):
                    tbatch(0, quad)
                    yield
                yield from quarter_po(q2T[(1, 0)], "wq", xq, 1, 0)
                yield from quarter_po(q2T[(1, 1)], "wq", xq, 1, 1)

            def weave_B():
                for qc in range(2):
                    half(k2T[(1, 0)], "wk", xt, 1, 0, qc)
                    yield
                for qc in range(2):
                    half(k2T[(1, 1)], "wk", xt, 1, 1, qc)
                    yield
                for qc in range(2):
                    half(v2T[(1, 0)], "wv", xt, 1, 0, qc)
                    yield
                for qc in range(2):
                    half(v2T[(1, 1)], "wv", xt, 1, 1, qc)
                    yield
                for quad in range(4):
                    tbatch(1, quad)
                    yield

            def weave_proj(b, qp, alt=False, act_ncol1=False):
                for pm in range(8):
                    for ncol in range(2):
                        use_sc = alt and (pm % 2 == 1)
                        proj_sub(
                            b, qp, pm, ncol,
                            use_act=(act_ncol1 and ncol == 1),
                            pool=(scp if use_sc else mp),
                            ptag=("sc" if use_sc else "m"),
                        )
                        yield

            def unit(b, qp, prev, gen, steps=1):
                for m in range(16):
                    scores(b, qp, m)
                    if prev is not None:
                        attnv(*prev, m)
                    if m >= 1:
                        denom(b, qp, m - 1)
                    for _ in range(steps):
                        next(gen, None)
                denom(b, qp, 15)
                if prev is not None:
                    po_evict(*prev)
                recip(b, qp)
                if prev is not None:
                    mul(*prev)
                shuffles(b, qp)

            # ---- S0: minimal prefix for scores(0,qp0,m<8) -----------------
            for _ in quarter_po(q2T[(0, 0)], "wq", xq, 0, 0):
                pass
            half(k2T[(0, 0)], "wk", xt, 0, 0, 0)
            half(k2T[(0, 0)], "wk", xt, 0, 0, 1)
            late_consts()

            # ---- four score units -----------------------------------------
            unit(0, 0, None, weave_A())
            unit(0, 1, (0, 0), weave_B())
            unit(1, 0, (0, 1), weave_proj(0, 0))
            unit(1, 1, (1, 0), weave_proj(0, 1))

            # ---- tail: attnv(1,1) + proj(1,0), then proj(1,1) -------------
            genE = weave_proj(1, 0, alt=True, act_ncol1=True)
            for m in range(16):
                attnv(1, 1, m)
                next(genE, None)
            po_evict(1, 1)
            mul(1, 1)
            for _ in weave_proj(1, 1, alt=True, act_ncol1=True):
                pass
    return nc


def _get_compiled():
    global _COMPILED_NC
    if _COMPILED_NC is None:
        nc = bacc.Bacc(
            "TRN2", target_bir_lowering=False, debug=False, num_devices=NCORES
        )
        _emit(nc)
        nc.compile()
        _COMPILED_NC = nc
    return _COMPILED_NC


def _install_trace_shim():
    """Register antenv.axon_hooks NTFF hook (missing on this image)."""
    import contextlib
    import ctypes
    import types

    if "antenv.axon_hooks" in sys.modules:
        return
    try:
        import antenv
    except ImportError:
        return
    so_path = "/opt/axon/libaxon_pjrt.so"
    if not os.path.exists(so_path):
        return

    mod = types.ModuleType("antenv.axon_hooks")
    mod._hook = None
    mod.set_axon_ntff_profile_hook = lambda h: setattr(mod, "_hook", h)
    mod.get_axon_ntff_profile_hook = lambda: mod._hook

    lib = ctypes.CDLL(so_path)
    if not hasattr(lib, "axon_start_nrt_profile"):
        return
    lib.axon_start_nrt_profile.argtypes = [
        ctypes.POINTER(ctypes.c_int64),
        ctypes.c_size_t,
    ]
    lib.axon_start_nrt_profile.restype = ctypes.c_int64
    lib.axon_stop_nrt_profile.argtypes = [ctypes.c_char_p]
    lib.axon_stop_nrt_profile.restype = ctypes.c_int64

    @contextlib.contextmanager
    def _hook(output_dir, device_ids):
        import jax

        jax.devices()
        if device_ids:
            ids = (ctypes.c_int64 * len(device_ids))(*device_ids)
            rc = lib.axon_start_nrt_profile(ids, len(device_ids))
        else:
            rc = lib.axon_start_nrt_profile(None, 0)
        if rc != 0:
            raise RuntimeError(f"axon_start_nrt_profile rc={rc}")
        try:
            yield
        finally:
            n = lib.axon_stop_nrt_profile(str(output_dir).encode())
            if n < 0:
                raise RuntimeError(f"axon_stop_nrt_profile rc={n}")

    mod.set_axon_ntff_profile_hook(_hook)
    sys.modules["antenv.axon_hooks"] = mod
    antenv.axon_hooks = mod


def kernel(query, target, Wq, Wk, Wv, Wp, bp):
    global LAST_EXEC_NS, LAST_RESULTS
    query = np.asarray(query, dtype=np.float32)
    target = np.asarray(target, dtype=np.float32)
    Wq = np.asarray(Wq, dtype=np.float32)
    Wk = np.asarray(Wk, dtype=np.float32)
    Wv = np.asarray(Wv, dtype=np.float32)
    Wp = np.asarray(Wp, dtype=np.float32)
    bp = np.asarray(bp, dtype=np.float32)

    xq = np.ascontiguousarray(query.reshape(B * N, C).T).astype(np.float16)
    xt = np.ascontiguousarray(target.reshape(B * N, C).T).astype(np.float16)
    id128 = np.eye(128, dtype=np.float16)

    def wlayout(Wm, rows):
        # SBUF weight tile [p, kc*128 + m] = W[row0 + m, kc*128 + p]
        ws = Wm[rows, :].astype(np.float16)  # (128, 1024)
        return np.ascontiguousarray(
            ws.reshape(128, 8, 128).transpose(2, 1, 0).reshape(128, 1024)
        )

    in_maps = []
    for c in range(NCORES):
        rows = slice(c * 128, (c + 1) * 128)
        in_maps.append(
            {
                "xq": xq,
                "xt": xt,
                "wq": wlayout(Wq, rows),
                "wk": wlayout(Wk, rows),
                "wv": wlayout(Wv, rows),
                "wp": np.ascontiguousarray(Wp[:, rows].T).astype(np.float16),
                "id128": id128,
            }
        )

    if TRACE:
        _install_trace_shim()

    nc = _get_compiled()
    res = run_bass_kernel_spmd(
        nc, in_maps, core_ids=list(range(NCORES)), trace=TRACE
    )
    LAST_RESULTS = res
    LAST_EXEC_NS = res.exec_time_ns

    acc = res.results[0]["out_p"].astype(np.float64)
    for c in range(1, NCORES):
        acc += res.results[c]["out_p"]
    out = acc.astype(np.float32) + bp[None, None, :]
    return out


# revision 12
# speedup vs baseline: 1.2231x; 1.2231x over previous
"""Trainium2 Bass kernel for nn_CrossAttention (B=2, N=2048, C=1024, H=16).

Sharding: 16 heads / 8 cores = 2 heads per core (both batches on every
core).  Each core computes its heads' Q/K/V projections with the matching
128-row slice of Wq/Wk/Wv, full attention for its 4 (batch, head) pairs,
and a partial output projection against its 128-column slice of Wp.  The
host sums the 8 partial projections (the tensor-parallel all-reduce) and
adds the bias.

v2 schedule: the ScalarE exp of 16.8M score elements (~147us at
[128,1024] grain) is the pacing engine; everything else is organized so
ScalarE never waits.  The kernel runs four "score units" (b, qp), each
16 m-iterations of: 4 scores matmuls (both heads, PE row-tiled) -> 2 exps
-> (delayed-one-unit) attn@V col-tiled pairs -> (lagged-1) denominator
quad -> one woven misc op (qkv projection half / V transpose batch /
output-projection piece).

PSUM map (8 banks of 2KB/partition):
  sc pool  2 x [128,1024] f32 = 4 banks  (scores ping-pong)
  o  pool  1 x [128,1024] f32 = 2 banks  (attnv accum; qkv quarters in A)
  d  pool  1 x [128, 512] f32 = 1 bank   (softmax denominators)
  m  pool  1 x [128, 512] f32 = 1 bank   (qkv halves / transposes / proj)

On-device layouts (per core, fp16 matmul operands):
  xq/xt    [1024 ch, 4096 pos]  channel-major inputs (host pre-transposed)
  q2T/k2T/v2T  4 x [128, 1024]  per (batch, pos-half); rows 0-63 head0
  vpos     2 x [128, 2048]      per batch: [keys-in-chunk, chunk*128 + hd]
  st       [128 keys, 2048]     exp(scores^T) per (b, qp, m): h0 | h1
  outT     4 x [128, 1024]      normalized attention output per (b, qp)
  out_p    [2, 2048, 1024] f32  partial projection (summed on host)
"""

import os
import sys

for _p in ("/opt/trn_rl_repo", os.path.expanduser("~/.axon_site/_ro/trn_rl_repo")):
    if os.path.isdir(_p) and _p not in sys.path:
        sys.path.insert(0, _p)

import numpy as np

import concourse.bacc as bacc
import concourse.mybir as mybir
import concourse.tile as tile
from concourse.bass_utils import run_bass_kernel_spmd

F16 = mybir.dt.float16
F32 = mybir.dt.float32
AF = mybir.ActivationFunctionType

B, N, C, H, D = 2, 2048, 1024, 16, 64
NCORES = 8
SCALE = float(D) ** -0.5

TRACE = False
LAST_EXEC_NS = None
LAST_RESULTS = None

_COMPILED_NC = None


def _emit(nc):
    xq = nc.dram_tensor("xq", [C, B * N], F16, kind="ExternalInput")
    xt = nc.dram_tensor("xt", [C, B * N], F16, kind="ExternalInput")
    wq = nc.dram_tensor("wq", [128, 1024], F16, kind="ExternalInput")
    wk = nc.dram_tensor("wk", [128, 1024], F16, kind="ExternalInput")
    wv = nc.dram_tensor("wv", [128, 1024], F16, kind="ExternalInput")
    wp = nc.dram_tensor("wp", [128, C], F16, kind="ExternalInput")
    id128 = nc.dram_tensor("id128", [128, 128], F16, kind="ExternalInput")
    outp = nc.dram_tensor("out_p", [B, N, C], F32, kind="ExternalOutput")

    with tile.TileContext(nc) as tc:
        with (
            tc.tile_pool(name="consts", bufs=1) as cpool,
            tc.tile_pool(name="xs", bufs=16) as xs,
            tc.tile_pool(name="big", bufs=1) as big,
            tc.tile_pool(name="stp", bufs=18) as stp,
            tc.tile_pool(name="ob", bufs=1) as obp,
            tc.tile_pool(name="rcp", bufs=2) as rcpool,
            tc.tile_pool(name="evo", bufs=2) as evop,
            tc.tile_pool(name="pev", bufs=4) as pevp,
            tc.tile_pool(name="sc", bufs=2, space="PSUM") as scp,
            tc.tile_pool(name="o", bufs=1, space="PSUM") as op,
            tc.tile_pool(name="d", bufs=1, space="PSUM") as dp,
            tc.tile_pool(name="m", bufs=1, space="PSUM") as mp,
        ):
            # ---- constants ------------------------------------------------
            w_sb = {}
            for name, dram in (("wq", wq), ("wk", wk), ("wv", wv)):
                t_ = cpool.tile([128, 1024], F16, tag=name, name=f"w_{name}")
                nc.sync.dma_start(t_[:], dram[:])
                w_sb[name] = t_
            wp_sb = cpool.tile([128, C], F16, tag="wp")
            id_sb = cpool.tile([128, 128], F16, tag="id128")
            ones_sb = cpool.tile([128, 1], F16, tag="ones")
            nc.vector.memset(ones_sb[:], 1.0)

            def late_consts():
                nc.sync.dma_start(id_sb[:], id128[:])
                nc.sync.dma_start(wp_sb[:], wp[:])

            q2T = {}
            k2T = {}
            v2T = {}
            vpos = {}
            outT = {}
            for b in range(2):
                vpos[b] = big.tile([128, 2048], F16, tag=f"vpos{b}", name=f"vpos{b}")
                for hf in range(2):
                    q2T[(b, hf)] = big.tile([128, 1024], F16, tag=f"q{b}{hf}", name=f"q2T{b}{hf}")
                    k2T[(b, hf)] = big.tile([128, 1024], F16, tag=f"k{b}{hf}", name=f"k2T{b}{hf}")
                    v2T[(b, hf)] = big.tile([128, 1024], F16, tag=f"v{b}{hf}", name=f"v2T{b}{hf}")
                    outT[(b, hf)] = obp.tile(
                        [128, 1024], F16, tag=f"outT{b}{hf}", name=f"outT{b}{hf}"
                    )

            st = {(b, qp): [] for b in range(2) for qp in range(2)}
            ps_o = {}
            ps_d = {}
            rc_t = {}

            # ---- emission helpers -----------------------------------------
            def xchunks(xdram, b, hf, eng, tagp):
                # 8 x [128,1024] input chunks for one (b, pos-half), issued
                # up-front so the matmuls that chase them never wait.  The
                # engine queue is a parameter to spread DMA-issue cost.
                ts = []
                c0 = b * 2048 + hf * 1024
                for kc in range(8):
                    x_t = xs.tile([128, 1024], F16, tag="x", name=f"x{tagp}{kc}")
                    eng.dma_start(
                        x_t[:], xdram[kc * 128 : (kc + 1) * 128, c0 : c0 + 1024]
                    )
                    ts.append(x_t)
                return ts

            def half_mp(dst, wname, chunks, qc):
                # one [128 rows, 512 pos] qkv projection half via the misc pool
                ps = mp.tile([128, 512], F32, tag="m", name=f"h{wname}{qc}")
                for kc in range(8):
                    nc.tensor.matmul(
                        ps[:],
                        lhsT=w_sb[wname][:, kc * 128 : (kc + 1) * 128],
                        rhs=chunks[kc][:, qc * 512 : (qc + 1) * 512],
                        start=(kc == 0),
                        stop=(kc == 7),
                    )
                nc.vector.tensor_copy(dst[:, qc * 512 : (qc + 1) * 512], ps[:])

            def quarter_po_g(dst, wname, chunks):
                # full [128,1024] quarter through the attnv slot (idle
                # before unit B); generator with 2 yield points so it can
                # be woven one ~1.7us chunk per m-iteration.
                ps = op.tile([128, 1024], F32, tag="o", name=f"qt{wname}")
                for kc in range(8):
                    for qc in range(2):
                        nc.tensor.matmul(
                            ps[:, qc * 512 : (qc + 1) * 512],
                            lhsT=w_sb[wname][:, kc * 128 : (kc + 1) * 128],
                            rhs=chunks[kc][:, qc * 512 : (qc + 1) * 512],
                            start=(kc == 0),
                            stop=(kc == 7),
                        )
                    if kc == 3:
                        yield
                nc.vector.tensor_copy(dst[:], ps[:])
                yield

            def tbatch(b, quad):
                # transpose 4 key-chunks of v2T into vpos via PE
                ps = mp.tile([128, 512], F16, tag="m", name=f"t{b}{quad}")
                for i in range(4):
                    ch = quad * 4 + i
                    src = v2T[(b, ch // 8)]
                    ks = slice((ch % 8) * 128, (ch % 8 + 1) * 128)
                    nc.tensor.transpose(
                        ps[:, i * 128 : (i + 1) * 128], src[:, ks], id_sb[:]
                    )
                nc.vector.tensor_copy(
                    vpos[b][:, quad * 512 : (quad + 1) * 512], ps[:]
                )

            def scores(b, qp, m):
                th = []
                for h in range(2):
                    th.append(
                        scp.tile([128, 1024], F32, tag="sc", name=f"sc{b}{qp}{m}{h}")
                    )
                kt = k2T[(b, m // 8)]
                ms = slice((m % 8) * 128, (m % 8 + 1) * 128)
                for qc in range(2):
                    cs = slice(qc * 512, (qc + 1) * 512)
                    for h in range(2):
                        hp = slice(h * 64, (h + 1) * 64)
                        nc.tensor.matmul(
                            th[h][:, cs],
                            lhsT=kt[hp, ms],
                            rhs=q2T[(b, qp)][hp, cs],
                            start=True,
                            stop=True,
                        )
                s = stp.tile([128, 2048], F16, tag="st", name=f"st{b}{qp}{m}")
                for h in range(2):
                    nc.scalar.activation(
                        s[:, h * 1024 : (h + 1) * 1024], th[h][:], AF.Exp, scale=SCALE
                    )
                st[(b, qp)].append(s)

            def attnv(b, qp, m):
                if m == 0:
                    ps_o[(b, qp)] = op.tile([128, 1024], F32, tag="o", name=f"o{b}{qp}")
                po = ps_o[(b, qp)]
                s = st[(b, qp)][m]
                kw = dict(start=(m == 0), stop=(m == 15))
                for qc in range(2):
                    cs = slice(qc * 512, (qc + 1) * 512)
                    for h in range(2):
                        nc.tensor.matmul(
                            po[h * 64 : (h + 1) * 64, cs],
                            lhsT=vpos[b][:, m * 128 + h * 64 : m * 128 + (h + 1) * 64],
                            rhs=s[:, h * 1024 + qc * 512 : h * 1024 + (qc + 1) * 512],
                            **kw,
                        )

            def denom(b, qp, m):
                if m == 0:
                    ps_d[(b, qp)] = dp.tile([128, 512], F32, tag="d", name=f"d{b}{qp}")
                pd = ps_d[(b, qp)]
                s = st[(b, qp)][m]
                kw = dict(start=(m == 0), stop=(m == 15))
                for qc in range(2):
                    for h in range(2):
                        row = h * 32 + qc * 64
                        nc.tensor.matmul(
                            pd[row : row + 1, 0:512],
                            lhsT=ones_sb[:, 0:1],
                            rhs=s[:, h * 1024 + qc * 512 : h * 1024 + (qc + 1) * 512],
                            skip_group_check=True,
                            tile_position=(0, row),
                            **kw,
                        )

            def recip(b, qp):
                # frees the pd slot for the next unit's denominators
                rc = rcpool.tile([128, 1024], F32, tag="rc", name=f"rc{b}{qp}")
                rc_t[(b, qp)] = rc
                nc.vector.reciprocal_approx_fast(rc[:, 0:512], ps_d[(b, qp)][:, 0:512])

            def shuffles(b, qp):
                # broadcast per-(h,qc) reciprocal rows across their 64-row spans
                rc = rc_t[(b, qp)]
                bcast = [0] * 32
                c0 = slice(0, 512)
                c1 = slice(512, 1024)
                for dst, srow in ((0, 64), (32, 64), (64, 96), (96, 96)):
                    nc.vector.stream_shuffle(
                        rc[dst : dst + 32, c1], rc[srow : srow + 32, c0], bcast
                    )
                for dst, srow in ((64, 32), (96, 32), (32, 0), (0, 0)):
                    nc.vector.stream_shuffle(
                        rc[dst : dst + 32, c0], rc[srow : srow + 32, c0], bcast
                    )

            def po_evict(b, qp):
                ev = evop.tile([128, 1024], F32, tag="evo", name=f"evo{b}{qp}")
                nc.vector.tensor_copy(ev[:], ps_o[(b, qp)][:])
                ps_o[(b, qp)] = ev

            def mul(b, qp):
                nc.vector.tensor_mul(
                    outT[(b, qp)][:], ps_o[(b, qp)][:], rc_t[(b, qp)][:]
                )

            def proj_sub(b, qp, pm, ncol, use_act=False, pool=None, ptag="m"):
                # one [128 pos, 512 outcols] piece of the output projection
                ps = (pool or mp).tile([128, 512], F32, tag=ptag, name=f"pp{b}{qp}{pm}{ncol}")
                nc.tensor.matmul(
                    ps[:],
                    lhsT=outT[(b, qp)][:, pm * 128 : (pm + 1) * 128],
                    rhs=wp_sb[:, ncol * 512 : (ncol + 1) * 512],
                    start=True,
                    stop=True,
                )
                ev = pevp.tile([128, 512], F32, tag="pev", name=f"pe{b}{qp}{pm}{ncol}")
                if use_act:
                    nc.scalar.copy(ev[:], ps[:])
                else:
                    nc.vector.tensor_copy(ev[:], ps[:])
                rows = slice(qp * 1024 + pm * 128, qp * 1024 + (pm + 1) * 128)
                nc.sync.dma_start(
                    outp[b, rows, ncol * 512 : (ncol + 1) * 512], ev[:]
                )

            def proj_big(b, qp, pm):
                # [128 pos, 1024 outcols] projection piece via the freed
                # scores slots (tail only); eviction split DVE/ACT
                ps = scp.tile([128, 1024], F32, tag="sc", name=f"pb{b}{qp}{pm}")
                for ncol in range(2):
                    nc.tensor.matmul(
                        ps[:, ncol * 512 : (ncol + 1) * 512],
                        lhsT=outT[(b, qp)][:, pm * 128 : (pm + 1) * 128],
                        rhs=wp_sb[:, ncol * 512 : (ncol + 1) * 512],
                        start=True,
                        stop=True,
                    )
                ev = pevp.tile([128, 1024], F32, tag="pev", name=f"pb{b}{qp}{pm}e")
                nc.vector.tensor_copy(ev[:, 0:512], ps[:, 0:512])
                nc.scalar.copy(ev[:, 512:1024], ps[:, 512:1024])
                rows = slice(qp * 1024 + pm * 128, qp * 1024 + (pm + 1) * 128)
                nc.sync.dma_start(outp[b, rows, :], ev[:])

            # ---- weave generators -----------------------------------------
            def weave_A(s0_xt):
                # attnv slot is idle in unit A: qkv quarters run through it
                yield from quarter_po_g(v2T[(0, 0)], "wv", s0_xt)
                xq01 = xchunks(xq, 0, 1, nc.gpsimd, "q01")
                yield from quarter_po_g(q2T[(0, 1)], "wq", xq01)
                xt01 = xchunks(xt, 0, 1, nc.gpsimd, "t01")
                yield from quarter_po_g(k2T[(0, 1)], "wk", xt01)
                yield from quarter_po_g(v2T[(0, 1)], "wv", xt01)
                for quad in range(4):
                    tbatch(0, quad)
                    yield
                xq10 = xchunks(xq, 1, 0, nc.gpsimd, "q10")
                yield from quarter_po_g(q2T[(1, 0)], "wq", xq10)
                xq11 = xchunks(xq, 1, 1, nc.gpsimd, "q11")
                yield from quarter_po_g(q2T[(1, 1)], "wq", xq11)

            def weave_B():
                xt10 = xchunks(xt, 1, 0, nc.gpsimd, "t10")
                for qc in range(2):
                    half_mp(k2T[(1, 0)], "wk", xt10, qc)
                    yield
                for qc in range(2):
                    half_mp(v2T[(1, 0)], "wv", xt10, qc)
                    yield
                xt11 = xchunks(xt, 1, 1, nc.gpsimd, "t11")
                for qc in range(2):
                    half_mp(k2T[(1, 1)], "wk", xt11, qc)
                    yield
                for qc in range(2):
                    half_mp(v2T[(1, 1)], "wv", xt11, qc)
                    yield
                for quad in range(4):
                    tbatch(1, quad)
                    yield

            def weave_proj(b, qp):
                for pm in range(8):
                    for ncol in range(2):
                        proj_sub(b, qp, pm, ncol)
                        yield

            def unit(b, qp, prev, gen, steps=1):
                for m in range(16):
                    scores(b, qp, m)
                    if prev is not None:
                        attnv(*prev, m)
                    if m >= 1:
                        denom(b, qp, m - 1)
                    for _ in range(steps):
                        next(gen, None)
                denom(b, qp, 15)
                if prev is not None:
                    po_evict(*prev)
                recip(b, qp)
                if prev is not None:
                    mul(*prev)
                shuffles(b, qp)

            # ---- S0: minimal prefix for scores(0,qp0,m<8) -----------------
            s0_xq = xchunks(xq, 0, 0, nc.sync, "q00")
            s0_xt = xchunks(xt, 0, 0, nc.gpsimd, "t00")
            for _ in quarter_po_g(q2T[(0, 0)], "wq", s0_xq):
                pass
            half_mp(k2T[(0, 0)], "wk", s0_xt, 0)
            half_mp(k2T[(0, 0)], "wk", s0_xt, 1)
            late_consts()

            # ---- four score units -----------------------------------------
            unit(0, 0, None, weave_A(s0_xt))
            unit(0, 1, (0, 0), weave_B())
            unit(1, 0, (0, 1), weave_proj(0, 0))
            unit(1, 1, (1, 0), weave_proj(0, 1))

            # ---- tail: attnv(1,1) + proj(1,0), then proj(1,1) -------------
            pms = iter(range(8))
            for m in range(16):
                attnv(1, 1, m)
                if m % 2 == 1:
                    proj_big(1, 0, next(pms))
            po_evict(1, 1)
            mul(1, 1)
            for pm in range(8):
                proj_big(1, 1, pm)
    return nc


def _get_compiled():
    global _COMPILED_NC
    if _COMPILED_NC is None:
        nc = bacc.Bacc(
            "TRN2", target_bir_lowering=False, debug=False, num_devices=NCORES
        )
        _emit(nc)
        nc.compile()
        _COMPILED_NC = nc
    return _COMPILED_NC


def _install_trace_shim():
    """Register antenv.axon_hooks NTFF hook (missing on this image)."""
    import contextlib
    import ctypes
    import types

    if "antenv.axon_hooks" in sys.modules:
        return
    try:
        import antenv
    except ImportError:
        return
    so_path = "/opt/axon/libaxon_pjrt.so"
    if not os.path.exists(so_path):
        return

    mod = types.ModuleType("antenv.axon_hooks")
    mod._hook = None
    mod.set_axon_ntff_profile_hook = lambda h: setattr(mod, "_hook", h)
    mod.get_axon_ntff_profile_hook = lambda: mod._hook

    lib = ctypes.CDLL(so_path)
    if not hasattr(lib, "axon_start_nrt_profile"):
        return
    lib.axon_start_nrt_profile.argtypes = [
        ctypes.POINTER(ctypes.c_int64),
        ctypes.c_size_t,
    ]
    lib.axon_start_nrt_profile.restype = ctypes.c_int64
    lib.axon_stop_nrt_profile.argtypes = [ctypes.c_char_p]
    lib.axon_stop_nrt_profile.restype = ctypes.c_int64

    @contextlib.contextmanager
    def _hook(output_dir, device_ids):
        import jax

        jax.devices()
        if device_ids:
            ids = (ctypes.c_int64 * len(device_ids))(*device_ids)
            rc = lib.axon_start_nrt_profile(ids, len(device_ids))
        else:
            rc = lib.axon_start_nrt_profile(None, 0)
        if rc != 0:
            raise RuntimeError(f"axon_start_nrt_profile rc={rc}")
        try:
            yield
        finally:
            n = lib.axon_stop_nrt_profile(str(output_dir).encode())
            if n < 0:
                raise RuntimeError(f"axon_stop_nrt_profile rc={n}")

    mod.set_axon_ntff_profile_hook(_hook)
    sys.modules["antenv.axon_hooks"] = mod
    antenv.axon_hooks = mod


def kernel(query, target, Wq, Wk, Wv, Wp, bp):
    global LAST_EXEC_NS, LAST_RESULTS
    query = np.asarray(query, dtype=np.float32)
    target = np.asarray(target, dtype=np.float32)
    Wq = np.asarray(Wq, dtype=np.float32)
    Wk = np.asarray(Wk, dtype=np.float32)
    Wv = np.asarray(Wv, dtype=np.float32)
    Wp = np.asarray(Wp, dtype=np.float32)
    bp = np.asarray(bp, dtype=np.float32)

    xq = np.ascontiguousarray(query.reshape(B * N, C).T).astype(np.float16)
    xt = np.ascontiguousarray(target.reshape(B * N, C).T).astype(np.float16)
    id128 = np.eye(128, dtype=np.float16)

    def wlayout(Wm, rows):
        # SBUF weight tile [p, kc*128 + m] = W[row0 + m, kc*128 + p]
        ws = Wm[rows, :].astype(np.float16)  # (128, 1024)
        return np.ascontiguousarray(
            ws.reshape(128, 8, 128).transpose(2, 1, 0).reshape(128, 1024)
        )

    in_maps = []
    for c in range(NCORES):
        rows = slice(c * 128, (c + 1) * 128)
        in_maps.append(
            {
                "xq": xq,
                "xt": xt,
                "wq": wlayout(Wq, rows),
                "wk": wlayout(Wk, rows),
                "wv": wlayout(Wv, rows),
                "wp": np.ascontiguousarray(Wp[:, rows].T).astype(np.float16),
                "id128": id128,
            }
        )

    if TRACE:
        _install_trace_shim()

    nc = _get_compiled()
    res = run_bass_kernel_spmd(
        nc, in_maps, core_ids=list(range(NCORES)), trace=TRACE
    )
    LAST_RESULTS = res
    LAST_EXEC_NS = res.exec_time_ns

    acc = res.results[0]["out_p"].astype(np.float64)
    for c in range(1, NCORES):
        acc += res.results[c]["out_p"]
    out = acc.astype(np.float32) + bp[None, None, :]
    return out
